# revision 47
# baseline (speedup 1.0000x reference)
"""Trainium2 Bass kernel for LocalLuongAttention.

reference semantics (B=32, S=4096, D=1024, O=1024, STDDEV=8):
    score[b,s]  = sum_d src[b,s,d] * tgt[b,d]
    weights     = softmax(score, axis=1) * exp(-(s-pos[b])^2 / (2*8^2))
    weighted[b] = sum_s weights[b,s] * src[b,s,:]
    out         = tanh(concat([tgt, weighted], 1) @ W)        # W: [2048, 1024]

Distribution: the attention (scores/softmax/weighted sum) is data-parallel
over batch, 4 batches per core on 8 cores.  The projection is
column-parallel: each core holds only W[:, 128c:128(c+1)] (1 MB instead of
8 MB) and computes out[:, 128c:128(c+1)] for ALL 32 batches; the tiny
per-batch weighted vectors (4 KB fp32) are exchanged with one AllGather
per batch into a Shared scratchpad, pipelined under the next batch's
streaming with all read-backs deferred to the end.  The host stitches the
column blocks and undoes the (b_local, core)-major batch ordering.

All scores run in bf16 on the PE: the host casts src to bf16 and
transposes it to [D, S] layout, ROTATED per batch so the 256-row window
around pos always occupies the fixed columns [S-256, S).  Scores
accumulate over 8 d-chunks into [1, 512] PSUM blocks (stationary tgt
column, streaming src rows) and land on partition 0 as one [1, 4096] row
whose tail 256 columns ARE the window scores -- so the softmax
normalizer, max, and window weights all live on partition 0 with no
cross-partition reductions or broadcasts at all.  The Gaussian decay is
<= exp(-32) outside +/-64 of pos, so the weighted sum only needs those
256 rows (host-sliced row-major in bf16); the bf16 window weights row is
transposed to partitions with two trivial PE matmuls against a ones
column.  Validated against the fp32 reference: rel err 7.4e-3 (< 2e-2).
"""

import sys

for _p in ("/opt/trn_rl_repo",):
    if _p not in sys.path:
        sys.path.insert(0, _p)

from contextlib import ExitStack

import ml_dtypes
import numpy as np

import concourse.bass as bass
import concourse.tile as tile
from concourse import bacc, bass_isa, mybir
from concourse._compat import with_exitstack
from concourse.bass_utils import run_bass_kernel_spmd

B, S, D, O = 32, 4096, 1024, 1024
STDDEV = 8.0
N_CORES = 8
BPC = B // N_CORES   # batches per core
WIN = 256            # window rows (always the last 256 stream columns)
HALF = 64            # guaranteed covered half-window
KC = (2 * D) // 128  # 16 contraction chunks of the projection
KD = D // 128        # 8 d-chunks of the score contraction
NB = S // 512        # 512-wide PE matmul blocks per batch (8)
HB = NB // 2         # blocks per half (PSUM-bank limited)
OSH = O // N_CORES   # output columns per core

FP32 = mybir.dt.float32
BF16 = mybir.dt.bfloat16

_CACHE = {}
LAST_RESULTS = None  # BassKernelResults of the most recent run

# device batch-column order is (b_local, core)-major: column 8*b + c holds
# global batch 4*c + b
PERM = [4 * (gg % N_CORES) + gg // N_CORES for gg in range(B)]


def _install_ntff_shim():
    """Register the NTFF profile hook that this image's antenv lacks."""
    import contextlib
    import ctypes
    import types

    if "antenv.axon_hooks" in sys.modules:
        return
    lib = ctypes.CDLL("/opt/axon/libaxon_pjrt.so")
    if not hasattr(lib, "axon_start_nrt_profile"):
        raise RuntimeError("libaxon_pjrt.so lacks profile symbols")
    lib.axon_start_nrt_profile.argtypes = [
        ctypes.POINTER(ctypes.c_int64), ctypes.c_size_t]
    lib.axon_start_nrt_profile.restype = ctypes.c_int64
    lib.axon_stop_nrt_profile.argtypes = [ctypes.c_char_p]
    lib.axon_stop_nrt_profile.restype = ctypes.c_int64

    @contextlib.contextmanager
    def _hook(output_dir, device_ids):
        import jax
        jax.devices()
        if device_ids:
            ids = (ctypes.c_int64 * len(device_ids))(*device_ids)
            rc = lib.axon_start_nrt_profile(ids, len(device_ids))
        else:
            rc = lib.axon_start_nrt_profile(None, 0)
        if rc != 0:
            raise RuntimeError(f"axon_start_nrt_profile rc={rc}")
        try:
            yield
        finally:
            n = lib.axon_stop_nrt_profile(str(output_dir).encode())
            print(f"ntff profile: {n} file(s) -> {output_dir}",
                  file=sys.stderr)

    m = types.ModuleType("antenv.axon_hooks")
    m.get_axon_ntff_profile_hook = lambda: _hook
    m.set_axon_ntff_profile_hook = lambda h: None
    sys.modules["antenv.axon_hooks"] = m
    import concourse.bass_utils as _bu
    _bu.upload_artifacts = lambda tmpdir: f"local://{tmpdir}"


@with_exitstack
def _body(ctx: ExitStack, tc: tile.TileContext, out, srcT, tgt16t, tgtall,
          srcwin16, logpw, wshard, wt_all):
    nc = tc.nc
    maxop = mybir.AluOpType.max
    addop = mybir.AluOpType.add
    Exp = mybir.ActivationFunctionType.Exp
    Tanh = mybir.ActivationFunctionType.Tanh
    Copy = mybir.ActivationFunctionType.Copy

    consts = ctx.enter_context(tc.tile_pool(name="consts", bufs=1))
    wpool = ctx.enter_context(tc.tile_pool(name="wpool", bufs=1))
    srcp = ctx.enter_context(tc.tile_pool(name="srcp", bufs=6))
    winp = ctx.enter_context(tc.tile_pool(name="winp", bufs=2))
    scp = ctx.enter_context(tc.tile_pool(name="scores", bufs=2))
    stats = ctx.enter_context(tc.tile_pool(name="stats", bufs=4))
    outp = ctx.enter_context(tc.tile_pool(name="outp", bufs=2))
    psc = ctx.enter_context(tc.tile_pool(name="psc", bufs=5, space="PSUM"))
    psw = ctx.enter_context(tc.tile_pool(name="psw", bufs=2, space="PSUM"))
    pso = ctx.enter_context(tc.tile_pool(name="pso", bufs=1, space="PSUM"))
    dram = ctx.enter_context(tc.tile_pool(name="dram", bufs=2, space="DRAM"))

    # Stationary tgt columns for the PE score matmuls: [128, d_chunk, batch]
    tg16 = consts.tile([128, KD, BPC], BF16)
    nc.sync.dma_start(out=tg16, in_=tgt16t.rearrange("(c p) b -> p c b",
                                                     p=128))
    ones16 = consts.tile([1, 1], BF16)
    nc.vector.memset(ones16, 1.0)

    # Resident projection weight shard W[:, 128c:128(c+1)]: [128, k, 128]
    wsb = wpool.tile([128, KC, OSH], FP32)

    # combined.T for ALL batches in (b_local, core)-major column order,
    # laid out [128, batch, k_chunk] so the per-batch all-gather read-back
    # is one contiguous DMA: combAt holds the tgt half (host-prearranged),
    # combAw the all-gathered weighted half.
    combAt = consts.tile([128, B, KD], FP32)
    combAw = consts.tile([128, B, KD], FP32)

    po = pso.tile([B, OSH], FP32)
    zdisc = consts.tile([1, 512], FP32)  # discarded exp output

    # Per-batch state carried from the stream phase to the (software-
    # pipelined) epilogue phases.
    bstate = {}

    def stream_phase(b):
        # stream DMAs: 4 x 2 MB [128, 2 chunks, S] bf16; window rows and
        # log-posweights interleave on the same queue
        srcr_ = srcT[b].rearrange("(g p) s -> p g s", p=128)
        sts = [srcp.tile([128, 2, S], BF16, name=f"st{q}", tag="st")
               for q in range(4)]
        if b == 0:
            # halve the first transfer so the PE's first tile lands sooner
            nc.sync.dma_start(out=sts[0][:, 0:1, :], in_=srcr_[:, 0:1, :])
            nc.sync.dma_start(out=sts[0][:, 1:2, :], in_=srcr_[:, 1:2, :])
        else:
            nc.sync.dma_start(out=sts[0], in_=srcr_[:, 0:2, :])
        nc.sync.dma_start(out=sts[1], in_=srcr_[:, 2:4, :])
        nc.sync.dma_start(out=sts[2], in_=srcr_[:, 4:6, :])
        nc.sync.dma_start(out=sts[3], in_=srcr_[:, 6:8, :])
        # window rows / log-posweights are consumed only by the (late)
        # epilogue -- load them behind the stream tiles
        win16 = winp.tile([128, 2, D], BF16)
        nc.sync.dma_start(out=win16,
                          in_=srcwin16[b].rearrange("(t p) d -> p t d",
                                                    p=128))
        lpw = stats.tile([1, WIN], FP32, tag="lpw")
        nc.sync.dma_start(out=lpw, in_=logpw[b:b + 1, :])

        # scores[0, s] = sum_d srcT[d, s] * tgt[d] on the PE, two halves
        # of 4 [1, 512] PSUM blocks each; the scalar engine copies blocks
        # to SBUF, the DVE reduces each half's max.
        scores = scp.tile([1, S], FP32)
        m8 = stats.tile([1, NB], FP32, tag="m8")
        negm8 = stats.tile([1, NB], FP32, tag="negm8")
        zb8 = stats.tile([1, NB], FP32, tag="zb8")
        for h in range(2):
            ps = [psc.tile([1, 512], FP32, name=f"ps{j}", tag="ps")
                  for j in range(HB)]
            for c in range(KD):
                st = sts[c // 2]
                for jj in range(HB):
                    j = HB * h + jj
                    nc.tensor.matmul(ps[jj], lhsT=tg16[:, c, b:b + 1],
                                     rhs=st[:, c % 2, 512 * j:512 * (j + 1)],
                                     start=(c == 0), stop=(c == KD - 1),
                                     skip_group_check=True)
            for jj in range(HB):
                j = HB * h + jj
                sl = scores[:, 512 * j:512 * (j + 1)]
                nc.scalar.activation(sl, ps[jj], Copy)
                nc.vector.tensor_reduce(m8[:, j:j + 1], sl,
                                        mybir.AxisListType.X, maxop)
                nc.vector.tensor_scalar_mul(negm8[:, j:j + 1],
                                            m8[:, j:j + 1], -1.0)
                nc.scalar.activation(zdisc, sl, Exp,
                                     bias=negm8[:, j:j + 1],
                                     accum_out=zb8[:, j:j + 1])
        bstate[b] = (win16, lpw, scores, m8, zb8)

    def epilogue_pre(b):
        # softmax stats + window weights, entirely on partition 0:
        # Z = sum_j zb_j * exp(mb_j - m) from the per-block partials
        (win16, lpw, scores, m8, zb8) = bstate[b]
        m1 = stats.tile([1, 1], FP32, tag="m1")
        nc.vector.tensor_reduce(m1, m8, mybir.AxisListType.X, maxop)
        negm = stats.tile([1, 1], FP32, tag="negm")
        nc.vector.tensor_scalar_mul(negm, m1, -1.0)
        e8 = stats.tile([1, NB], FP32, tag="e8")
        nc.scalar.activation(e8, m8, Exp, bias=negm)
        zz8 = stats.tile([1, NB], FP32, tag="zz8")
        zp = stats.tile([1, 1], FP32, tag="zp")
        nc.vector.scalar_tensor_tensor(
            out=zz8, in0=e8, scalar=0.0, in1=zb8,
            op0=mybir.AluOpType.bypass, op1=mybir.AluOpType.mult,
            accum_out=zp)
        rz = stats.tile([1, 1], FP32, tag="rz")
        nc.vector.reciprocal(rz, zp)
        # window weights: exp(score + logpw - m) / Z -> bf16 row [1, 256]
        wpre = stats.tile([1, WIN], FP32, tag="wpre")
        nc.vector.tensor_add(wpre, scores[:, S - WIN:S], lpw)
        wexp = stats.tile([1, WIN], FP32, tag="wexp")
        nc.scalar.activation(wexp, wpre, Exp, bias=negm)
        wfin = stats.tile([1, WIN], BF16, tag="wfin")
        nc.vector.tensor_scalar_mul(wfin, wexp, rz)
        bstate[b] = (win16, wfin)

    def epilogue_pe(b):
        (win16, wfin) = bstate.pop(b)
        # transpose the weights row to partitions: two matmuls vs ones
        wfinT = stats.tile([128, 2], BF16, tag="wfinT")
        for t in range(2):
            pwt = psw.tile([128, 1], FP32, tag="pw", name="pwt")
            nc.tensor.matmul(pwt, lhsT=wfin[:, 128 * t:128 * (t + 1)],
                             rhs=ones16, start=True, stop=True)
            nc.scalar.activation(wfinT[:, t:t + 1], pwt, Copy)
        # weighted.T chunks: contract window rows on the PE (bf16)
        combL = stats.tile([128, KD], FP32, name=f"combL{b}", tag="combL")
        for c in range(KD):
            pw = psw.tile([128, 1], FP32, tag="pw", name="pw")
            nc.tensor.matmul(pw, lhsT=win16[:, 0, 128 * c:128 * (c + 1)],
                             rhs=wfinT[:, 0:1], start=True, stop=False)
            nc.tensor.matmul(pw, lhsT=win16[:, 1, 128 * c:128 * (c + 1)],
                             rhs=wfinT[:, 1:2], start=False, stop=True)
            nc.scalar.activation(combL[:, c:c + 1], pw, Copy)

        # all-gather this batch's weighted vector into the Shared
        # scratchpad; overlaps the next batch's streaming.
        wt_loc = dram.tile([128, KD], FP32, name="wt_loc", tag="wt_loc")
        nc.gpsimd.dma_start(out=wt_loc, in_=combL)
        nc.gpsimd.collective_compute(
            "AllGather",
            mybir.AluOpType.bypass,
            replica_groups=[list(range(N_CORES))],
            ins=[wt_loc[:].opt()],
            outs=[wt_all[b].opt()],
        )

    for b in range(BPC):
        if b > 0:
            epilogue_pre(b - 1)
        stream_phase(b)
        if b == 0:
            # projection constants load behind batch 0's stream traffic
            nc.sync.dma_start(out=wsb,
                              in_=wshard.rearrange("(k p) n -> p k n",
                                                   p=128))
            nc.sync.dma_start(out=combAt, in_=tgtall)
        if b > 0:
            epilogue_pe(b - 1)
    epilogue_pre(BPC - 1)
    # tgt half of the projection: emitted BEFORE the last collective (a
    # collective acts as an all-engine barrier for later-emitted
    # instructions) so these matmuls fill the PE while batch 3's chain
    # and all-gather drain; the accumulation group closes below.
    for k in range(KC // 2):
        nc.tensor.matmul(po, lhsT=combAt[:, :, k], rhs=wsb[:, k, :],
                         start=(k == 0), stop=False,
                         skip_group_check=True)
    epilogue_pe(BPC - 1)
    for b in range(BPC):
        nc.sync.dma_start(
            out=combAw[:, N_CORES * b:N_CORES * (b + 1), :],
            in_=wt_all[b].rearrange("n p c -> p n c"))
    for k in range(KC // 2, KC):
        nc.tensor.matmul(po, lhsT=combAw[:, :, k - KC // 2],
                         rhs=wsb[:, k, :],
                         start=False, stop=(k == KC - 1),
                         skip_group_check=True)
    ot = outp.tile([B, OSH], FP32)
    nc.scalar.activation(ot, po, Tanh)
    nc.sync.dma_start(out=out, in_=ot)


def build():
    if "nc" in _CACHE:
        return _CACHE["nc"]
    nc = bacc.Bacc("TRN2", target_bir_lowering=False, debug=False,
                   enable_asserts=False, num_devices=N_CORES)
    srcT = nc.dram_tensor("srcT", [BPC, D, S], BF16,
                          kind="ExternalInput").ap()
    tgt16t = nc.dram_tensor("tgt16t", [D, BPC], BF16,
                            kind="ExternalInput").ap()
    tgtall = nc.dram_tensor("tgtall", [128, B, KD], FP32,
                            kind="ExternalInput").ap()
    srcwin16 = nc.dram_tensor("srcwin16", [BPC, WIN, D], BF16,
                              kind="ExternalInput").ap()
    logpw = nc.dram_tensor("logpw", [BPC, WIN], FP32,
                           kind="ExternalInput").ap()
    wshard = nc.dram_tensor("wshard", [2 * D, OSH], FP32,
                            kind="ExternalInput").ap()
    out = nc.dram_tensor("out", [B, OSH], FP32, kind="ExternalOutput").ap()
    wt_all = nc.dram_tensor("wt_all_sh", [BPC, N_CORES, 128, KD], FP32,
                            kind="Internal", addr_space="Shared").ap()
    with tile.TileContext(nc) as tc:
        _body(tc, out, srcT, tgt16t, tgtall, srcwin16, logpw, wshard, wt_all)
    nc.compile()
    _CACHE["nc"] = nc
    return nc


def make_in_maps(src, tgt, pos, wmat):
    """Host-side sharding: bf16 cast, per-batch rotation so the window is
    always the last 256 stream columns, transpose to [D, S]."""
    src16 = src.astype(ml_dtypes.bfloat16)
    tgt16 = tgt.astype(ml_dtypes.bfloat16)
    tgtall = np.ascontiguousarray(
        tgt[PERM].reshape(B, KD, 128).transpose(2, 0, 1))
    w0 = np.clip(128 * ((pos.astype(np.int64) - HALF) // 128), 0, S - WIN)
    j_idx = np.arange(WIN, dtype=np.int64)[None, :]
    in_maps = []
    for c in range(N_CORES):
        bsl = slice(c * BPC, (c + 1) * BPC)
        srcT = np.stack([
            np.roll(src16[c * BPC + i], (S - WIN) - w0[c * BPC + i],
                    axis=0).T
            for i in range(BPC)
        ])
        srcwin16 = np.stack([
            src16[c * BPC + i, w0[c * BPC + i]:w0[c * BPC + i] + WIN, :]
            for i in range(BPC)
        ])
        logpw = (
            -((w0[bsl][:, None] + j_idx - pos[bsl][:, None])
              .astype(np.float64) ** 2) / (2.0 * STDDEV * STDDEV)
        ).astype(np.float32)
        in_maps.append({
            "srcT": np.ascontiguousarray(srcT),
            "tgt16t": np.ascontiguousarray(tgt16[bsl].T),
            "tgtall": tgtall,
            "srcwin16": np.ascontiguousarray(srcwin16),
            "logpw": logpw,
            "wshard": np.ascontiguousarray(wmat[:, c * OSH:(c + 1) * OSH]),
        })
    return in_maps


def kernel(source_hidden_sequence, target_hidden, positions,
           attention_weights, trace=False):
    src = np.ascontiguousarray(source_hidden_sequence, dtype=np.float32)
    tgt = np.ascontiguousarray(target_hidden, dtype=np.float32)
    pos = np.asarray(positions)
    wmat = np.ascontiguousarray(attention_weights, dtype=np.float32)
    assert src.shape == (B, S, D) and wmat.shape == (2 * D, O)

    nc = build()
    if trace:
        _install_ntff_shim()
    in_maps = make_in_maps(src, tgt, pos, wmat)
    res = run_bass_kernel_spmd(nc, in_maps, list(range(N_CORES)), trace=trace)
    global LAST_RESULTS
    LAST_RESULTS = res
    outs = np.concatenate([res.results[c]["out"] for c in range(N_CORES)],
                          axis=1)
    # undo the (b_local, core)-major device row order
    full = np.empty_like(outs)
    full[PERM] = outs
    return full.astype(np.float32)
